# revision 9
# baseline (speedup 1.0000x reference)
"""Trainium2 Bass kernel for nn_DecoderLayer (moe_routing).

Strategy (8 NeuronCores, SPMD, 4 launches; host does only data movement
and integer routing between launches):

  L1  token-sharded projections: self Q/K/V for each 512-token shard,
      cross K/V for each 512-row encoder shard.            (all 8 cores)
  L2  token-sharded attention core: self-attn (causal, full keys via
      host-assembled K/V), GLN1, cross-attn, GLN2, router logits,
      softmax top-2 + gates + prob sums.                   (all 8 cores)
  L3  expert-sharded MoE FFN: core e runs expert e over the tokens
      routed to it (host gathers tokens into a capacity-C batch).
  L4  token-sharded combine: h2 + c1 + c2, GLN3.

Token shard i = (batch i//4, rows (i%4)*512 ... +512).
All heavy matmuls run in fp16 (fp32 PSUM accumulation); residual
stream, norms and router run in fp32 (float32r matmul for the router).
"""

import os
import numpy as np
import ml_dtypes

import concourse.bass as bass
from concourse import bacc
import concourse.tile as tile
import concourse.mybir as mybir
from concourse import bass_utils
from concourse import masks

F32 = mybir.dt.float32
F32R = mybir.dt.float32r
F16 = mybir.dt.float16
AF = mybir.ActivationFunctionType
ALU = mybir.AluOpType
AX = mybir.AxisListType

B, T, S, D, H = 2, 2048, 2048, 1024, 16
DH = D // H          # 64
E, F, G, K = 8, 4096, 4, 2
EPS = 1e-5
NC = 8
SH = 512             # tokens per core
CT = D // 128        # 8 channel tiles

_cache = {}

def _load_split(nc, dst_tile, dram_ap, inner):
    """DMA (G*128, inner) DRAM tensor into SBUF tile (128, G*inner),
    group g at [:, g*inner:(g+1)*inner]."""
    nc.sync.dma_start(
        dst_tile.rearrange("p (g q) -> p g q", q=inner),
        dram_ap.rearrange("(g p) q -> p g q", p=128))



# Collect per-launch exec times when tracing is enabled (test.py reads this).
LAST_EXEC_NS = []
_TRACE = bool(int(os.environ.get("BASSKERNEL_TRACE", "0")))


def _run(nc, in_maps, label):
    trace = _TRACE
    res = bass_utils.run_bass_kernel_spmd(nc, in_maps, core_ids=list(range(NC)),
                                          trace=trace)
    if trace:
        LAST_EXEC_NS.append((label, res.exec_time_ns))
    return res.results


def f16(x):
    return np.ascontiguousarray(x, dtype=np.float16)


def f32(x):
    return np.ascontiguousarray(x, dtype=np.float32)


# ===================================================================== L1
def build_l1():
    if "l1" in _cache:
        return _cache["l1"]
    nc = bacc.Bacc(trn_type="TRN2", target_bir_lowering=False, debug=False,
                   num_devices=NC)
    xT = nc.dram_tensor("xT", [D, SH], F16, kind="ExternalInput")
    encT = nc.dram_tensor("encT", [D, SH], F16, kind="ExternalInput")
    ws = {}
    for w in ("wq_s", "wk_s", "wv_s", "wk_c", "wv_c"):
        ws[w] = nc.dram_tensor(w, [D, D], F16, kind="ExternalInput")
    qsT = nc.dram_tensor("qsT", [D, SH], F16, kind="ExternalOutput")
    ksT = nc.dram_tensor("ksT", [D, SH], F16, kind="ExternalOutput")
    vs = nc.dram_tensor("vs", [SH, D], F16, kind="ExternalOutput")
    kcT = nc.dram_tensor("kcT", [D, SH], F16, kind="ExternalOutput")
    vc = nc.dram_tensor("vc", [SH, D], F16, kind="ExternalOutput")

    with tile.TileContext(nc) as tc:
        with tc.tile_pool(name="act", bufs=1) as act, \
             tc.tile_pool(name="wpool", bufs=2) as wp, \
             tc.tile_pool(name="opool", bufs=3) as op, \
             tc.tile_pool(name="ps", bufs=3, space="PSUM") as ps:
            xt = act.tile([128, CT * SH], F16, tag="xt")      # x^T c-tiles
            et = act.tile([128, CT * SH], F16, tag="et")
            _load_split(nc, xt[:], xT.ap(), SH)
            _load_split(nc, et[:], encT.ap(), SH)

            def proj_T(wname, src, out_dram):
                # out^T[d, q] = sum_c W[c, d] * src^T[c, q]
                wt = wp.tile([128, CT * D], F16, tag="w")
                _load_split(nc, wt[:], ws[wname].ap(), D)
                for dt in range(CT):
                    pt = ps.tile([128, SH], F32, tag="p")
                    for ct in range(CT):
                        nc.tensor.matmul(
                            pt[:], wt[:, ct * D + dt * 128:ct * D + (dt + 1) * 128],
                            src[:, ct * SH:(ct + 1) * SH],
                            start=(ct == 0), stop=(ct == CT - 1))
                    ot = op.tile([128, SH], F16, tag="o")
                    nc.scalar.copy(ot[:], pt[:])
                    nc.sync.dma_start(out_dram.ap()[dt * 128:(dt + 1) * 128, :], ot[:])

            def proj_N(wname, src, out_dram):
                # out[q, d] = sum_c src^T[c, q] * W[c, d]  (token-major)
                wt = wp.tile([128, CT * D], F16, tag="w")
                _load_split(nc, wt[:], ws[wname].ap(), D)
                for qc in range(SH // 128):
                    for dc in range(D // 512):
                        pt = ps.tile([128, 512], F32, tag="p")
                        for ct in range(CT):
                            nc.tensor.matmul(
                                pt[:],
                                src[:, ct * SH + qc * 128:ct * SH + (qc + 1) * 128],
                                wt[:, ct * D + dc * 512:ct * D + (dc + 1) * 512],
                                start=(ct == 0), stop=(ct == CT - 1))
                        ot = op.tile([128, 512], F16, tag="o")
                        nc.scalar.copy(ot[:], pt[:])
                        nc.sync.dma_start(
                            out_dram.ap()[qc * 128:(qc + 1) * 128,
                                          dc * 512:(dc + 1) * 512], ot[:])

            proj_T("wq_s", xt, qsT)
            proj_T("wk_s", xt, ksT)
            proj_N("wv_s", xt, vs)
            proj_T("wk_c", et, kcT)
            proj_N("wv_c", et, vc)
    nc.compile()
    _cache["l1"] = nc
    return nc


# ===================================================================== L2
def build_l2():
    if "l2" in _cache:
        return _cache["l2"]
    nc = bacc.Bacc(trn_type="TRN2", target_bir_lowering=False, debug=False,
                   num_devices=NC)
    qsT = nc.dram_tensor("qsT", [D, SH], F16, kind="ExternalInput")
    ks = nc.dram_tensor("ks", [H, DH, T], F16, kind="ExternalInput")
    vsa = nc.dram_tensor("vsa", [H, T, DH + 1], F16, kind="ExternalInput")
    kc = nc.dram_tensor("kc", [H, DH, S], F16, kind="ExternalInput")
    vca = nc.dram_tensor("vca", [H, S, DH + 1], F16, kind="ExternalInput")
    xr = nc.dram_tensor("xr", [SH, D], F32, kind="ExternalInput")
    cmask = nc.dram_tensor("cmask", [T, SH], F16, kind="ExternalInput")
    g1w = nc.dram_tensor("g1w", [SH, D], F32, kind="ExternalInput")
    g1b = nc.dram_tensor("g1b", [SH, D], F32, kind="ExternalInput")
    g2w = nc.dram_tensor("g2w", [SH, D], F32, kind="ExternalInput")
    g2b = nc.dram_tensor("g2b", [SH, D], F32, kind="ExternalInput")
    wo_s = nc.dram_tensor("wo_s", [D, D], F16, kind="ExternalInput")
    wq_c = nc.dram_tensor("wq_c", [D, D], F16, kind="ExternalInput")
    wo_c = nc.dram_tensor("wo_c", [D, D], F16, kind="ExternalInput")
    wr = nc.dram_tensor("wr", [D, E], F32, kind="ExternalInput")
    h2o = nc.dram_tensor("h2o", [SH, D], F32, kind="ExternalOutput")
    h2To = nc.dram_tensor("h2To", [D, SH], F16, kind="ExternalOutput")
    route = nc.dram_tensor("route", [SH, 4], F32, kind="ExternalOutput")
    prsum = nc.dram_tensor("prsum", [1, E], F32, kind="ExternalOutput")

    with tile.TileContext(nc) as tc:
        with tc.tile_pool(name="kv", bufs=2) as kvp, \
             tc.tile_pool(name="qp", bufs=1) as qp, \
             tc.tile_pool(name="pt", bufs=4) as ptp, \
             tc.tile_pool(name="ot", bufs=1) as otp, \
             tc.tile_pool(name="hs", bufs=1) as hsp, \
             tc.tile_pool(name="bw", bufs=2) as bwp, \
             tc.tile_pool(name="sm", bufs=3) as smp, \
             tc.tile_pool(name="msk", bufs=2) as mskp, \
             tc.tile_pool(name="np_", bufs=2) as npp, \
             tc.tile_pool(name="sps", bufs=2, space="PSUM") as spsp, \
             tc.tile_pool(name="avps", bufs=1, space="PSUM") as avpsp, \
             tc.tile_pool(name="bcps", bufs=1, space="PSUM") as bcpsp, \
             tc.tile_pool(name="opps", bufs=2, space="PSUM") as oppsp, \
             tc.tile_pool(name="prps", bufs=1, space="PSUM") as prpsp:

            ones_bc = smp.tile([64, 64], F16, tag="ones_bc")
            nc.gpsimd.memset(ones_bc[:], 1.0)
            ident = smp.tile([128, 128], F32, tag="ident")
            masks.make_identity(nc, ident[:])
            epst = smp.tile([128, 1], F32, tag="epst")
            nc.gpsimd.memset(epst[:], EPS)

            def attention(qT_t, k_dram, va_dram, klen, masked, out_oT):
                """qT_t: SBUF tile (64, H*SH) fp16, head h at [:, h*SH:].
                out_oT: SBUF tile (128, CT*SH) fp16, channel-major attn out
                (normalized); head pair m lands in c-tile m."""
                nkt = klen // 128
                for m in range(H // 2):
                    avp = avpsp.tile([128, SH], F32, tag="av")
                    denp = avpsp.tile([64, SH], F32, tag="den", bufs=1)
                    for hh in range(2):
                        h = 2 * m + hh
                        kh = kvp.tile([64, klen], F16, tag="kh")
                        nc.sync.dma_start(kh[:], k_dram.ap()[h])
                        vh = kvp.tile([128, nkt * (DH + 1)], F16, tag="vh")
                        _load_split(nc, vh[:], va_dram.ap()[h], DH + 1)
                        qh = qT_t[:, h * SH:(h + 1) * SH]
                        for kt in range(nkt):
                            sp = spsp.tile([128, SH], F32, tag="s")
                            nc.tensor.matmul(sp[:], kh[:, kt * 128:(kt + 1) * 128],
                                             qh, start=True, stop=True)
                            pt = ptp.tile([128, SH], F16, tag="p")
                            nc.scalar.activation(pt[:], sp[:], AF.Exp, scale=0.125)
                            if masked:
                                mt = mskp.tile([128, SH], F16, tag="m")
                                nc.sync.dma_start(
                                    mt[:], cmask.ap()[kt * 128:(kt + 1) * 128, :])
                                nc.vector.tensor_mul(pt[:], pt[:], mt[:])
                            nc.tensor.matmul(
                                avp[hh * 64:hh * 64 + 64, :],
                                vh[:, kt * (DH + 1):kt * (DH + 1) + DH],
                                pt[:],
                                start=(kt == 0), stop=(kt == nkt - 1),
                                tile_position=(0, hh * 64))
                            nc.tensor.matmul(
                                denp[hh * 32:hh * 32 + 1, :],
                                vh[:, kt * (DH + 1) + DH:(kt + 1) * (DH + 1)],
                                pt[:],
                                start=(kt == 0), stop=(kt == nkt - 1),
                                tile_position=(0, hh * 32))
                    den_sb = smp.tile([64, SH], F32, tag="den_sb")
                    rden16 = smp.tile([64, SH], F16, tag="rden16")
                    bcp = bcpsp.tile([128, SH], F32, tag="bc")
                    for hh in range(2):
                        r = hh * 32
                        nc.vector.tensor_copy(den_sb[r:r + 1, :], denp[r:r + 1, :])
                        nc.vector.reciprocal(den_sb[r:r + 1, :], den_sb[r:r + 1, :])
                        nc.vector.tensor_copy(rden16[r:r + 1, :], den_sb[r:r + 1, :])
                        nc.tensor.matmul(
                            bcp[hh * 64:hh * 64 + 64, :],
                            ones_bc[r:r + 1, :],
                            rden16[r:r + 1, :],
                            start=True, stop=True,
                            tile_position=(r, hh * 64))
                    bc_sb = smp.tile([128, SH], F16, tag="bc_sb")
                    nc.scalar.copy(bc_sb[:], bcp[:])
                    nc.vector.tensor_tensor(
                        out=out_oT[:, m * SH:(m + 1) * SH], in0=avp[:],
                        in1=bc_sb[:], op=ALU.mult)

            def oproj_resid_gln(oT, w_dram, resid_tiles, gw_dram, gb_dram,
                                out_tiles):
                """token-major out = GLN(resid + oT @ W). oT (128, CT*SH) fp16
                channel-major; resid_tiles/out_tiles: 4 tiles (128, D) f32."""
                wt = bwp.tile([128, CT * D], F16, tag="w")
                _load_split(nc, wt[:], w_dram.ap(), D)
                for qc in range(SH // 128):
                    h1 = out_tiles[qc]
                    for dc in range(D // 512):
                        pt = oppsp.tile([128, 512], F32, tag="op")
                        for ct in range(CT):
                            nc.tensor.matmul(
                                pt[:],
                                oT[:, ct * SH + qc * 128:ct * SH + (qc + 1) * 128],
                                wt[:, ct * D + dc * 512:ct * D + (dc + 1) * 512],
                                start=(ct == 0), stop=(ct == CT - 1))
                        nc.vector.tensor_add(
                            h1[:, dc * 512:(dc + 1) * 512],
                            resid_tiles[qc][:, dc * 512:(dc + 1) * 512], pt[:])
                    # layernorm over D (free dim)
                    s1 = smp.tile([128, 1], F32, tag="s1")
                    nc.vector.reduce_sum(s1[:], h1[:], axis=AX.X)
                    sq = smp.tile([128, 1], F32, tag="sq")
                    sqscr = npp.tile([128, D], F32, tag="sqscr")
                    nc.scalar.activation(sqscr[:], h1[:], AF.Square,
                                         accum_out=sq[:])
                    mu = smp.tile([128, 1], F32, tag="mu")
                    nc.scalar.mul(mu[:], s1[:], 1.0 / D)
                    var = smp.tile([128, 1], F32, tag="var")
                    nc.vector.tensor_mul(var[:], mu[:], mu[:])
                    nc.vector.scalar_tensor_tensor(
                        out=var[:], in0=sq[:], scalar=1.0 / D, in1=var[:],
                        op0=ALU.mult, op1=ALU.subtract)
                    std = smp.tile([128, 1], F32, tag="std")
                    nc.scalar.activation(std[:], var[:], AF.Sqrt, bias=epst[:])
                    rstd = smp.tile([128, 1], F32, tag="rstd")
                    nc.vector.reciprocal(rstd[:], std[:])
                    gw = npp.tile([128, D], F32, tag="gw")
                    nc.sync.dma_start(gw[:], gw_dram.ap()[qc * 128:(qc + 1) * 128, :])
                    gb = npp.tile([128, D], F32, tag="gb")
                    nc.sync.dma_start(gb[:], gb_dram.ap()[qc * 128:(qc + 1) * 128, :])
                    nc.vector.tensor_scalar(out=h1[:], in0=h1[:], scalar1=mu[:],
                                            scalar2=rstd[:], op0=ALU.subtract,
                                            op1=ALU.mult)
                    nc.vector.tensor_mul(h1[:], h1[:], gw[:])
                    nc.vector.tensor_add(h1[:], h1[:], gb[:])

            # ---------------- self attention
            qsT_t = qp.tile([64, H * SH], F16, tag="qsT")
            nc.sync.dma_start(
                qsT_t[:].rearrange("p (h q) -> p h q", q=SH),
                qsT.ap().rearrange("(h p) q -> p h q", p=64))
            oT1 = otp.tile([128, CT * SH], F16, tag="oT")
            attention(qsT_t, ks, vsa, T, True, oT1)

            x_tiles = []
            for qc in range(SH // 128):
                xt_ = hsp.tile([128, D], F32, tag=f"xr{qc}", name=f"xr_{qc}")
                nc.sync.dma_start(xt_[:], xr.ap()[qc * 128:(qc + 1) * 128, :])
                x_tiles.append(xt_)
            h1_tiles = [hsp.tile([128, D], F32, tag=f"h1{i}", name=f"h1_{i}") for i in range(4)]
            oproj_resid_gln(oT1, wo_s, x_tiles, g1w, g1b, h1_tiles)

            # h1^T (channel-major fp16) via PE transpose
            h1T = otp.tile([128, CT * SH], F16, tag="h1T")
            for qc in range(SH // 128):
                for ctile in range(CT):
                    tp = oppsp.tile([128, 128], F32, tag="op")
                    nc.tensor.transpose(
                        tp[:], h1_tiles[qc][:, ctile * 128:(ctile + 1) * 128],
                        ident[:])
                    nc.scalar.copy(
                        h1T[:, ctile * SH + qc * 128:ctile * SH + (qc + 1) * 128],
                        tp[:])

            # cross Q projection: qcT[d, q] = sum_c Wq_c[c, d] h1T[c, q]
            qcT_t = qp.tile([64, H * SH], F16, tag="qcT")
            wt = bwp.tile([128, CT * D], F16, tag="w")
            _load_split(nc, wt[:], wq_c.ap(), D)
            for h in range(H):
                pt = oppsp.tile([64, SH], F32, tag="op", name=f"opq_{h}")
                for ctile in range(CT):
                    nc.tensor.matmul(
                        pt[:],
                        wt[:, ctile * D + h * 64:ctile * D + h * 64 + 64],
                        h1T[:, ctile * SH:(ctile + 1) * SH],
                        start=(ctile == 0), stop=(ctile == CT - 1))
                nc.scalar.copy(qcT_t[:, h * SH:(h + 1) * SH], pt[:])

            # ---------------- cross attention
            oT2 = otp.tile([128, CT * SH], F16, tag="oT2")
            attention(qcT_t, kc, vca, S, False, oT2)
            h2_tiles = [hsp.tile([128, D], F32, tag=f"h2{i}", name=f"h2_{i}") for i in range(4)]
            oproj_resid_gln(oT2, wo_c, h1_tiles, g2w, g2b, h2_tiles)
            for qc in range(SH // 128):
                nc.sync.dma_start(h2o.ap()[qc * 128:(qc + 1) * 128, :],
                                  h2_tiles[qc][:])

            # ---------------- h2^T + router
            wrt = smp.tile([128, CT * E], F32, tag="wr")
            _load_split(nc, wrt[:], wr.ap(), E)
            iotaf = smp.tile([128, E], F32, tag="iotaf")
            iotai = smp.tile([128, E], mybir.dt.int32, tag="iotai")
            nc.gpsimd.iota(iotai[:], pattern=[[1, E]], base=0, channel_multiplier=0)
            nc.vector.tensor_copy(iotaf[:], iotai[:])
            ones_pr = smp.tile([128, 1], F32, tag="ones_pr")
            nc.gpsimd.memset(ones_pr[:], 1.0)
            prp = prpsp.tile([1, E], F32, tag="pr")

            for qc in range(SH // 128):
                h2T_f32 = npp.tile([128, D], F32, tag="h2T32")
                for ctile in range(CT):
                    tp = oppsp.tile([128, 128], F32, tag="op")
                    nc.tensor.transpose(
                        tp[:], h2_tiles[qc][:, ctile * 128:(ctile + 1) * 128],
                        ident[:])
                    nc.vector.tensor_copy(
                        h2T_f32[:, ctile * 128:(ctile + 1) * 128], tp[:])
                h2T16 = npp.tile([128, D], F16, tag="h2T16")
                nc.scalar.copy(h2T16[:], h2T_f32[:])
                nc.sync.dma_start(
                    h2To.ap().rearrange("(c p) q -> p c q", p=128)[:, :,
                        qc * 128:(qc + 1) * 128],
                    h2T16[:].rearrange("p (c q) -> p c q", q=128))

                lg = oppsp.tile([128, E], F32, tag="op", name=f"lg_{qc}")
                for ctile in range(CT):
                    nc.tensor.matmul(
                        lg[:],
                        h2T_f32[:, ctile * 128:(ctile + 1) * 128],
                        wrt[:, ctile * E:(ctile + 1) * E],
                        start=(ctile == 0), stop=(ctile == CT - 1))
                lgs = smp.tile([128, E], F32, tag="lgs")
                nc.vector.tensor_copy(lgs[:], lg[:])

                m1 = smp.tile([128, 1], F32, tag="m1")
                nc.vector.reduce_max(m1[:], lgs[:], axis=AX.X)
                eq = smp.tile([128, E], F32, tag="eq")
                nc.vector.tensor_scalar(out=eq[:], in0=lgs[:], scalar1=m1[:],
                                        scalar2=None, op0=ALU.is_ge)
                tmp = smp.tile([128, E], F32, tag="tmp")
                nc.vector.scalar_tensor_tensor(
                    out=tmp[:], in0=eq[:], scalar=-1024.0, in1=iotaf[:],
                    op0=ALU.mult, op1=ALU.add)
                idx1 = smp.tile([128, 1], F32, tag="idx1")
                nc.vector.tensor_reduce(out=idx1[:], in_=tmp[:], axis=AX.X,
                                        op=ALU.min)
                nc.vector.tensor_scalar_add(idx1[:], idx1[:], 1024.0)
                l2 = smp.tile([128, E], F32, tag="l2")
                nc.vector.scalar_tensor_tensor(
                    out=l2[:], in0=eq[:], scalar=-1e4, in1=lgs[:],
                    op0=ALU.mult, op1=ALU.add)
                m2 = smp.tile([128, 1], F32, tag="m2")
                nc.vector.reduce_max(m2[:], l2[:], axis=AX.X)
                eq2 = smp.tile([128, E], F32, tag="eq2")
                nc.vector.tensor_scalar(out=eq2[:], in0=l2[:], scalar1=m2[:],
                                        scalar2=None, op0=ALU.is_ge)
                nc.vector.scalar_tensor_tensor(
                    out=tmp[:], in0=eq2[:], scalar=-1024.0, in1=iotaf[:],
                    op0=ALU.mult, op1=ALU.add)
                idx2 = smp.tile([128, 1], F32, tag="idx2")
                nc.vector.tensor_reduce(out=idx2[:], in_=tmp[:], axis=AX.X,
                                        op=ALU.min)
                nc.vector.tensor_scalar_add(idx2[:], idx2[:], 1024.0)
                # gates
                d21 = smp.tile([128, 1], F32, tag="d21")
                nc.vector.tensor_sub(d21[:], m2[:], m1[:])
                e2t = smp.tile([128, 1], F32, tag="e2t")
                nc.scalar.activation(e2t[:], d21[:], AF.Exp)
                onep = smp.tile([128, 1], F32, tag="onep")
                nc.vector.tensor_scalar_add(onep[:], e2t[:], 1.0)
                g1t = smp.tile([128, 1], F32, tag="g1t")
                nc.vector.reciprocal(g1t[:], onep[:])
                g2t = smp.tile([128, 1], F32, tag="g2t")
                nc.vector.tensor_mul(g2t[:], e2t[:], g1t[:])
                rt = smp.tile([128, 4], F32, tag="rt")
                nc.vector.tensor_copy(rt[:, 0:1], idx1[:])
                nc.vector.tensor_copy(rt[:, 1:2], idx2[:])
                nc.vector.tensor_copy(rt[:, 2:3], g1t[:])
                nc.vector.tensor_copy(rt[:, 3:4], g2t[:])
                nc.sync.dma_start(route.ap()[qc * 128:(qc + 1) * 128, :], rt[:])
                # probs for aux
                pr = smp.tile([128, E], F32, tag="prob")
                negm = smp.tile([128, 1], F32, tag="negm")
                nc.vector.tensor_scalar_mul(negm[:], m1[:], -1.0)
                nc.scalar.activation(pr[:], lgs[:], AF.Exp, bias=negm[:])
                se = smp.tile([128, 1], F32, tag="se")
                nc.vector.reduce_sum(se[:], pr[:], axis=AX.X)
                rse = smp.tile([128, 1], F32, tag="rse")
                nc.vector.reciprocal(rse[:], se[:])
                nc.vector.tensor_scalar_mul(pr[:], pr[:], rse[:])
                nc.tensor.matmul(prp[:], ones_pr[:], pr[:], start=(qc == 0),
                                 stop=(qc == SH // 128 - 1))
            prs = smp.tile([1, E], F32, tag="prs")
            nc.vector.tensor_copy(prs[:], prp[:])
            nc.sync.dma_start(prsum.ap(), prs[:])
    nc.compile()
    _cache["l2"] = nc
    return nc


# ===================================================================== L3
def build_l3(C):
    key = ("l3", C)
    if key in _cache:
        return _cache[key]
    nc = bacc.Bacc(trn_type="TRN2", target_bir_lowering=False, debug=False,
                   num_devices=NC)
    hgT = nc.dram_tensor("hgT", [D, C], F16, kind="ExternalInput")
    w1 = nc.dram_tensor("w1", [D, F], F16, kind="ExternalInput")
    b1 = nc.dram_tensor("b1", [F, 1], F32, kind="ExternalInput")
    w2 = nc.dram_tensor("w2", [F, D], F16, kind="ExternalInput")
    b2 = nc.dram_tensor("b2", [1, D], F16, kind="ExternalInput")
    gates = nc.dram_tensor("gates", [C, 1], F32, kind="ExternalInput")
    contrib = nc.dram_tensor("contrib", [C, D], F16, kind="ExternalOutput")

    FT = F // 128   # 32 f-tiles
    # q chunks of up to 512 for mm1
    qchunks = []
    q0 = 0
    while q0 < C:
        qn = min(512, C - q0)
        qchunks.append((q0, qn))
        q0 += qn

    with tile.TileContext(nc) as tc:
        with tc.tile_pool(name="hg", bufs=1) as hgp, \
             tc.tile_pool(name="w1p", bufs=2) as w1p, \
             tc.tile_pool(name="w2p", bufs=1) as w2p, \
             tc.tile_pool(name="hm", bufs=1) as hmp, \
             tc.tile_pool(name="bia", bufs=1) as biap, \
             tc.tile_pool(name="ob", bufs=3) as obp, \
             tc.tile_pool(name="ps1", bufs=2, space="PSUM") as ps1p, \
             tc.tile_pool(name="ps2", bufs=4, space="PSUM") as ps2p:
            hg = hgp.tile([128, CT * C], F16, tag="hg")
            _load_split(nc, hg[:], hgT.ap(), C)
            b1t = biap.tile([128, FT], F32, tag="b1")
            _load_split(nc, b1t[:], b1.ap(), 1)
            b2t = biap.tile([1, D], F16, tag="b2")
            nc.sync.dma_start(b2t[:], b2.ap())
            ones1 = biap.tile([1, 128], F16, tag="ones1")
            nc.gpsimd.memset(ones1[:], 1.0)
            gt = biap.tile([128, C // 128], F32, tag="gt")
            _load_split(nc, gt[:], gates.ap(), 1)
            w2t = w2p.tile([128, FT * D], F16, tag="w2")
            _load_split(nc, w2t[:], w2.ap(), D)

            hm = hmp.tile([128, FT * C], F16, tag="hm")   # hidden^T (f, q)
            for ft in range(FT):
                w1t = w1p.tile([128, CT * 128], F16, tag="w1")
                nc.sync.dma_start(
                    w1t[:].rearrange("p (c f) -> p c f", f=128),
                    w1.ap().rearrange("(c p) f -> p c f", p=128)[:, :,
                        ft * 128:(ft + 1) * 128])
                for (q0, qn) in qchunks:
                    pt = ps1p.tile([128, 512], F32, tag="p1")
                    for ctile in range(CT):
                        nc.tensor.matmul(
                            pt[:, 0:qn],
                            w1t[:, ctile * 128:(ctile + 1) * 128],
                            hg[:, ctile * C + q0:ctile * C + q0 + qn],
                            start=(ctile == 0), stop=(ctile == CT - 1))
                    nc.scalar.activation(hm[:, ft * C + q0:ft * C + q0 + qn],
                                         pt[:, 0:qn], AF.Relu,
                                         bias=b1t[:, ft:ft + 1])
            # mm2: contrib[q, d] = relu_h^T @ W2 + b2, scaled by gates
            for qq in range(C // 128):
                for dc in range(D // 512):
                    pt = ps2p.tile([128, 512], F32, tag="p2")
                    for ft in range(FT):
                        nc.tensor.matmul(
                            pt[:],
                            hm[:, ft * C + qq * 128:ft * C + (qq + 1) * 128],
                            w2t[:, ft * D + dc * 512:ft * D + (dc + 1) * 512],
                            start=(ft == 0), stop=False)
                    nc.tensor.matmul(pt[:], ones1[:],
                                     b2t[:, dc * 512:(dc + 1) * 512],
                                     start=False, stop=True)
                    ot = obp.tile([128, 512], F16, tag="ob")
                    nc.scalar.activation(ot[:], pt[:], AF.Copy,
                                         scale=gt[:, qq:qq + 1])
                    nc.sync.dma_start(
                        contrib.ap()[qq * 128:(qq + 1) * 128,
                                     dc * 512:(dc + 1) * 512], ot[:])
    nc.compile()
    _cache[key] = nc
    return nc


# ===================================================================== L4
def build_l4():
    if "l4" in _cache:
        return _cache["l4"]
    nc = bacc.Bacc(trn_type="TRN2", target_bir_lowering=False, debug=False,
                   num_devices=NC)
    h2 = nc.dram_tensor("h2", [SH, D], F32, kind="ExternalInput")
    c1 = nc.dram_tensor("c1", [SH, D], F32, kind="ExternalInput")
    c2 = nc.dram_tensor("c2", [SH, D], F32, kind="ExternalInput")
    g3w = nc.dram_tensor("g3w", [SH, D], F32, kind="ExternalInput")
    g3b = nc.dram_tensor("g3b", [SH, D], F32, kind="ExternalInput")
    out = nc.dram_tensor("out", [SH, D], F32, kind="ExternalOutput")

    with tile.TileContext(nc) as tc:
        with tc.tile_pool(name="sb", bufs=2) as sb, \
             tc.tile_pool(name="sm", bufs=2) as sm:
            epst = sm.tile([128, 1], F32, tag="eps")
            nc.gpsimd.memset(epst[:], EPS)
            for qc in range(SH // 128):
                ht = sb.tile([128, D], F32, tag="h")
                nc.sync.dma_start(ht[:], h2.ap()[qc * 128:(qc + 1) * 128, :])
                c1t = sb.tile([128, D], F32, tag="c1")
                nc.sync.dma_start(c1t[:], c1.ap()[qc * 128:(qc + 1) * 128, :])
                c2t = sb.tile([128, D], F32, tag="c2")
                nc.sync.dma_start(c2t[:], c2.ap()[qc * 128:(qc + 1) * 128, :])
                nc.vector.tensor_add(ht[:], ht[:], c1t[:])
                nc.vector.tensor_add(ht[:], ht[:], c2t[:])
                s1 = sm.tile([128, 1], F32, tag="s1")
                nc.vector.reduce_sum(s1[:], ht[:], axis=AX.X)
                sq = sm.tile([128, 1], F32, tag="sq")
                scr = sb.tile([128, D], F32, tag="scr")
                nc.scalar.activation(scr[:], ht[:], AF.Square, accum_out=sq[:])
                mu = sm.tile([128, 1], F32, tag="mu")
                nc.scalar.mul(mu[:], s1[:], 1.0 / D)
                var = sm.tile([128, 1], F32, tag="var")
                nc.vector.tensor_mul(var[:], mu[:], mu[:])
                nc.vector.scalar_tensor_tensor(
                    out=var[:], in0=sq[:], scalar=1.0 / D, in1=var[:],
                    op0=ALU.mult, op1=ALU.subtract)
                std = sm.tile([128, 1], F32, tag="std")
                nc.scalar.activation(std[:], var[:], AF.Sqrt, bias=epst[:])
                rstd = sm.tile([128, 1], F32, tag="rstd")
                nc.vector.reciprocal(rstd[:], std[:])
                gw = sb.tile([128, D], F32, tag="gw")
                nc.sync.dma_start(gw[:], g3w.ap()[qc * 128:(qc + 1) * 128, :])
                gb = sb.tile([128, D], F32, tag="gb")
                nc.sync.dma_start(gb[:], g3b.ap()[qc * 128:(qc + 1) * 128, :])
                nc.vector.tensor_scalar(out=ht[:], in0=ht[:], scalar1=mu[:],
                                        scalar2=rstd[:], op0=ALU.subtract,
                                        op1=ALU.mult)
                nc.vector.tensor_mul(ht[:], ht[:], gw[:])
                nc.vector.tensor_add(ht[:], ht[:], gb[:])
                nc.sync.dma_start(out.ap()[qc * 128:(qc + 1) * 128, :], ht[:])
    nc.compile()
    _cache["l4"] = nc
    return nc


# ================================================================ host glue
def kernel(x, encoder_output, token_types,
           Wq_s, Wk_s, Wv_s, Wo_s,
           Wq_c, Wk_c, Wv_c, Wo_c,
           Wr, W1, b1, W2, b2,
           g1_w, g1_b, g2_w, g2_b, g3_w, g3_b):
    LAST_EXEC_NS.clear()
    x = f32(x); encoder_output = f32(encoder_output)
    token_types = np.asarray(token_types)

    # ---------------- L1: projections
    nc1 = build_l1()
    w16 = {n: f16(w) for n, w in (("wq_s", Wq_s), ("wk_s", Wk_s),
                                  ("wv_s", Wv_s), ("wk_c", Wk_c),
                                  ("wv_c", Wv_c))}
    in1 = []
    for i in range(NC):
        b, j = divmod(i, 4)
        xs = x[b, j * SH:(j + 1) * SH]                 # (512, 1024)
        es = encoder_output[b, j * SH:(j + 1) * SH]
        in1.append({"xT": f16(xs.T), "encT": f16(es.T), **w16})
    r1 = _run(nc1, in1, "L1")

    # assemble per-batch full K^T/V (head-major) with ones column on V
    ks_b, vsa_b, kc_b, vca_b, qsT_b = [], [], [], [], []
    for b in range(B):
        ksT = np.concatenate([r1[4 * b + j]["ksT"] for j in range(4)], axis=1)
        kcT = np.concatenate([r1[4 * b + j]["kcT"] for j in range(4)], axis=1)
        vs = np.concatenate([r1[4 * b + j]["vs"] for j in range(4)], axis=0)
        vc = np.concatenate([r1[4 * b + j]["vc"] for j in range(4)], axis=0)
        ks_b.append(np.ascontiguousarray(ksT.reshape(H, DH, T)))
        kc_b.append(np.ascontiguousarray(kcT.reshape(H, DH, S)))

        def augment(v, L):
            va = np.empty((H, L, DH + 1), np.float16)
            vv = v.reshape(L, H, DH)
            va[:, :, :DH] = vv.transpose(1, 0, 2)
            va[:, :, DH] = np.float16(1.0)
            return va
        vsa_b.append(augment(vs, T))
        vca_b.append(augment(vc, S))
        qsT_b.append([r1[4 * b + j]["qsT"] for j in range(4)])

    # causal masks (k, q) per core
    kidx = np.arange(T)[:, None]
    cmasks = []
    for j in range(4):
        qidx = np.arange(SH)[None, :] + j * SH
        cmasks.append(f16(qidx >= kidx))

    # per-token norm params
    def tokp(p, b, j):
        tt = token_types[b, j * SH:(j + 1) * SH]
        return f32(p[tt])

    # ---------------- L2: attention + router
    nc2 = build_l2()
    wo_s16, wq_c16, wo_c16 = f16(Wo_s), f16(Wq_c), f16(Wo_c)
    wr32 = f32(Wr)
    in2 = []
    for i in range(NC):
        b, j = divmod(i, 4)
        in2.append({
            "qsT": qsT_b[b][j], "ks": ks_b[b], "vsa": vsa_b[b],
            "kc": kc_b[b], "vca": vca_b[b],
            "xr": f32(x[b, j * SH:(j + 1) * SH]),
            "cmask": cmasks[j],
            "g1w": tokp(g1_w, b, j), "g1b": tokp(g1_b, b, j),
            "g2w": tokp(g2_w, b, j), "g2b": tokp(g2_b, b, j),
            "wo_s": wo_s16, "wq_c": wq_c16, "wo_c": wo_c16, "wr": wr32,
        })
    r2 = _run(nc2, in2, "L2")

    h2_full = np.concatenate([r2[i]["h2o"] for i in range(NC)], axis=0)  # (4096, D)
    h2T_full = np.concatenate([r2[i]["h2To"] for i in range(NC)], axis=1)  # (D, 4096)
    route = np.concatenate([r2[i]["route"] for i in range(NC)], axis=0)   # (4096, 4)
    prsums = np.stack([r2[i]["prsum"][0] for i in range(NC)])             # (8, E)

    idx = route[:, 0:2].astype(np.int64)      # (4096, 2)
    gats = route[:, 2:4].astype(np.float64)   # normalized gates

    # ---------------- routing (host: integer work + gather only)
    NTOK = B * T
    tok_lists, gate_lists = [], []
    for e in range(E):
        sel = np.nonzero(idx[:, 0] == e)[0]
        sel2 = np.nonzero(idx[:, 1] == e)[0]
        toks = np.concatenate([sel, sel2])
        gs = np.concatenate([gats[sel, 0], gats[sel2, 1]])
        tok_lists.append(toks)
        gate_lists.append(gs)
    maxc = max(len(t) for t in tok_lists)
    C = ((maxc + 127) // 128) * 128
    nc3 = build_l3(C)

    w1_16 = [f16(W1[e]) for e in range(E)]
    w2_16 = [f16(W2[e]) for e in range(E)]
    in3 = []
    for e in range(E):
        toks = tok_lists[e]
        hgT = np.zeros((D, C), np.float16)
        hgT[:, :len(toks)] = h2T_full[:, toks]
        g = np.zeros((C, 1), np.float32)
        g[:len(toks), 0] = gate_lists[e]
        in3.append({"hgT": hgT, "w1": w1_16[e],
                    "b1": f32(b1[e]).reshape(F, 1), "w2": w2_16[e],
                    "b2": f16(b2[e]).reshape(1, D), "gates": g})
    r3 = _run(nc3, in3, "L3")

    # scatter contributions back (host: pure reordering)
    c_by_rank = np.zeros((2, NTOK, D), np.float32)
    for e in range(E):
        toks = tok_lists[e]
        contrib = r3[e]["contrib"][:len(toks)].astype(np.float32)
        n1 = np.sum(idx[:, 0] == e)
        c_by_rank[0, toks[:n1]] = contrib[:n1]
        c_by_rank[1, toks[n1:]] = contrib[n1:]

    # ---------------- L4: combine + GLN3
    nc4 = build_l4()
    in4 = []
    for i in range(NC):
        b, j = divmod(i, 4)
        lo = i * SH
        in4.append({
            "h2": np.ascontiguousarray(h2_full[lo:lo + SH]),
            "c1": np.ascontiguousarray(c_by_rank[0, lo:lo + SH]),
            "c2": np.ascontiguousarray(c_by_rank[1, lo:lo + SH]),
            "g3w": tokp(g3_w, b, j), "g3b": tokp(g3_b, b, j),
        })
    r4 = _run(nc4, in4, "L4")
    h_out = np.concatenate([r4[i]["out"] for i in range(NC)], axis=0)
    h_out = h_out.reshape(B, T, D)

    # aux loss (16 scalar flops on host)
    counts = np.bincount(idx.reshape(-1), minlength=E).astype(np.float64)
    fvec = counts / NTOK / K
    P = prsums.sum(0) / NTOK
    aux = np.float32(E * np.sum(fvec * P))
    return h_out, aux
